# revision 7
# baseline (speedup 1.0000x reference)
"""Trainium2 Bass kernel v2 for nn_Block (moe_routing).

Sharding over 8 NeuronCores:
  - LN1 token-sharded (own 512 tokens) + AllGather h1.
  - Head-parallel causal attention: core c owns head-pair c (heads 2c,
    2c+1) over the FULL batch; upper-triangle key tiles skipped.
  - Output projection computed as partial sums over pairs, ReduceScatter
    (add) back to token shards.
  - LN2 + gate logits local; logits AllGathered separately (small, so
    routing overlaps the big h2 AllGather into the gather table).
  - Routing computed directly in the wrapped [16, N/16] layout that
    sparse_gather consumes (no DRAM round trips).
  - Expert-parallel MoE (core c = expert c), CAP=1280 slots, 5 chunks of
    256, double-buffered hidden tiles, weights preloaded early.
"""

import math
import os
import sys

import numpy as np

sys.path.insert(0, "/opt/trn_rl_repo")

import concourse.bass as bass  # noqa: E402
import concourse.tile as tile  # noqa: E402
from concourse import bacc, mybir  # noqa: E402
from concourse.alu_op_type import AluOpType  # noqa: E402
from concourse.masks import make_identity  # noqa: E402

AF = mybir.ActivationFunctionType
FP32 = mybir.dt.float32
BF16 = mybir.dt.bfloat16
I32 = mybir.dt.int32
I16 = mybir.dt.int16
FP16 = mybir.dt.float16
P = 128
NCORE = 8
EPS = 1e-5


class Cfg:
    def __init__(self):
        self.B = 2
        self.T = 2048
        self.D = 1024
        self.H = 16
        self.HD = 64
        self.F = 4096
        self.E = 8
        self.CAP = 1280
        self.MOE_CHUNK = 256
        self.N = self.B * self.T       # 4096 tokens
        self.TL = self.N // NCORE      # 512 tokens per core
        self.DC = self.D // P          # 8
        self.FT = self.F // P          # 32
        self.KTN = self.N // P         # 32 key tiles (global)
        self.KTB = self.T // P         # 16 key tiles per batch
        self.G = self.T // self.TL     # 4 query groups of 512 per batch
        self.TLT = self.TL // P        # 4
        self.CI = self.CAP // P        # 10
        self.CAP16 = self.CAP // 16    # 80
        self.NT16 = self.N // 16       # 256
        self.MCN = self.CAP // self.MOE_CHUNK   # 5
        self.MCT = self.MOE_CHUNK // P          # 2
        self.MC16 = self.MOE_CHUNK // 16        # 16


def _nslices(n, step=512):
    return [(i, min(step, n - i)) for i in range(0, n, step)]


def build_nc(cfg: Cfg):
    c = cfg
    nc = bacc.Bacc("TRN2", target_bir_lowering=False, debug=False,
                   num_devices=NCORE)
    RG = [list(range(NCORE))]

    def din(name, shape, dt=FP32):
        return nc.dram_tensor(name, list(shape), dt, kind="ExternalInput").ap()

    # ---------------- I/O ----------------
    xpqT = din("xpqT", (c.D, c.TL))           # (x + pos) own block, transposed
    xqT = din("xqT", (c.D, c.TL))             # x own block, transposed
    wq = din("wq", (P, c.DC, P), BF16)        # my pair, pre-scaled by D^-0.5
    wk = din("wk", (P, c.DC, P), BF16)
    wv = din("wv", (P, c.DC, P), BF16)
    wp = din("wp", (P, c.DC, P), BF16)        # wproj.T rows of my pair
    bproj = din("bproj", (c.D,))
    ln1g = din("ln1g", (c.D,))
    ln1b = din("ln1b", (c.D,))
    ln2g = din("ln2g", (c.D,))
    ln2b = din("ln2b", (c.D,))
    gwT = din("gwT", (c.D, c.E))
    w1 = din("w1", (c.FT, P, c.DC, P), BF16)  # my expert
    b1 = din("b1", (c.F,))
    w2 = din("w2", (c.F, c.D), BF16)
    b2 = din("b2", (c.D,))
    maskd = din("maskd", (P, c.TLT, c.TL), BF16)  # diag causal block masks
    onehot = din("onehot", (1, c.E))
    out = nc.dram_tensor("out", [c.TL, c.D], FP32, kind="ExternalOutput").ap()

    # ---------------- internal DRAM ----------------
    NPAD = c.N + 2 * P
    HT = c.TL // 2
    h1_inA = nc.dram_tensor("h1_inA", [c.D, HT], BF16).ap()
    h1_inB = nc.dram_tensor("h1_inB", [c.D, HT], BF16).ap()
    h1_fullA = nc.dram_tensor("h1_fullA", [NCORE, c.D, HT], BF16,
                              addr_space="Shared").ap()
    h1_fullB = nc.dram_tensor("h1_fullB", [NCORE, c.D, HT], BF16,
                              addr_space="Shared").ap()
    HD2 = c.D // 2
    x2pA = nc.dram_tensor("x2pA", [NCORE, HD2, c.TL], FP16).ap()
    x2pB = nc.dram_tensor("x2pB", [NCORE, HD2, c.TL], FP16).ap()
    x2psA = nc.dram_tensor("x2psA", [HD2, c.TL], FP16).ap()
    x2psB = nc.dram_tensor("x2psB", [HD2, c.TL], FP16).ap()
    lg_in = nc.dram_tensor("lg_in", [c.TL, 2 * c.E], BF16).ap()
    lg_full = nc.dram_tensor("lg_full", [c.N, 2 * c.E], BF16,
                             addr_space="Shared").ap()
    ag_h2_in = nc.dram_tensor("ag_h2_in", [c.TL, c.D], BF16).ap()
    h2_gt = nc.dram_tensor("h2_gt", [NPAD, c.D], BF16).ap()
    idx16_dram = nc.dram_tensor("idx16_dram", [16, c.CAP16], I16).ap()
    cwlin = nc.dram_tensor("cwlin", [c.CAP], FP32).ap()
    moe_full = nc.dram_tensor("moe_full", [NPAD, c.D], BF16).ap()
    moe_slice = nc.dram_tensor("moe_slice", [c.TL, c.D], BF16).ap()

    with tile.TileContext(nc) as tc:
        with tc.tile_pool(name="persist", bufs=1) as pp:
            # ---------------- constants ----------------
            ident = pp.tile([P, P], FP32)
            make_identity(nc, ident[:])
            identb = pp.tile([P, P], BF16)
            nc.vector.tensor_copy(out=identb[:], in_=ident[:])
            ones_bf = pp.tile([P, 1], BF16)
            nc.vector.memset(ones_bf[:], 1.0)

            def load_pcol(ap_dram, n):
                t = pp.tile([P, n], FP32, name=ap_dram.tensor.name + "_sb")
                nc.sync.dma_start(out=t[:], in_=ap_dram.rearrange(
                    "(a p) -> p a", p=P))
                return t

            ln1g_sb = load_pcol(ln1g, c.DC)
            ln1b_sb = load_pcol(ln1b, c.DC)

            # long-lived tiles
            x2r_all = pp.tile([P, c.TLT, c.D], FP32)   # x2 rows (residual)
            cw_all = pp.tile([P, c.CI], FP32)
            idx_w = pp.tile([P, c.CAP16], I16)

            # ---------------- LN helper (transposed layout) ----------------
            def ln_tr(ctx_pool, ps_pool, xp, ntok, g_sb, b_sb, out_t, odt):
                """LN over partition axis of xp [128, DC, ntok] fp32.
                Writes result into out_t [128, DC, ntok] dtype odt.
                xp is centered in place."""
                for ns, nn in _nslices(ntok):
                    ps_sum = ps_pool.tile([1, 512], FP32, tag="ln_ps1")
                    for dc in range(c.DC):
                        xb = ctx_pool.tile([P, 512], BF16, tag="ln_xb")
                        nc.vector.tensor_copy(out=xb[:, :nn],
                                              in_=xp[:, dc, ns:ns + nn])
                        nc.tensor.matmul(ps_sum[:, :nn], ones_bf[:],
                                         xb[:, :nn],
                                         start=(dc == 0), stop=(dc == c.DC - 1))
                    negmu = ctx_pool.tile([1, 512], FP32, tag="ln_negmu")
                    nc.vector.tensor_scalar(out=negmu[:, :nn],
                                            in0=ps_sum[:, :nn],
                                            scalar1=-1.0 / c.D, scalar2=None,
                                            op0=AluOpType.mult)
                    bc = ctx_pool.tile([P, 512], FP32, tag="ln_bc")
                    nc.gpsimd.partition_broadcast(out_ap=bc[:, :nn],
                                                  in_ap=negmu[:, :nn])
                    for dc in range(c.DC):
                        nc.vector.tensor_tensor(out=xp[:, dc, ns:ns + nn],
                                                in0=xp[:, dc, ns:ns + nn],
                                                in1=bc[:, :nn],
                                                op=AluOpType.add)
                    ps_sq = ps_pool.tile([1, 512], FP32, tag="ln_ps2")
                    for dc in range(c.DC):
                        sq = ctx_pool.tile([P, 512], BF16, tag="ln_sq")
                        nc.scalar.activation(out=sq[:, :nn],
                                             in_=xp[:, dc, ns:ns + nn],
                                             func=AF.Square)
                        nc.tensor.matmul(ps_sq[:, :nn], ones_bf[:],
                                         sq[:, :nn],
                                         start=(dc == 0), stop=(dc == c.DC - 1))
                    std = ctx_pool.tile([1, 512], FP32, tag="ln_std")
                    nc.vector.tensor_scalar(out=std[:, :nn],
                                            in0=ps_sq[:, :nn],
                                            scalar1=1.0 / c.D, scalar2=EPS,
                                            op0=AluOpType.mult,
                                            op1=AluOpType.add)
                    nc.scalar.activation(out=std[:, :nn], in_=std[:, :nn],
                                         func=AF.Sqrt)
                    rstd = ctx_pool.tile([1, 512], FP32, tag="ln_rstd")
                    nc.vector.reciprocal(out=rstd[:, :nn], in_=std[:, :nn])
                    bcs = ctx_pool.tile([P, 512], FP32, tag="ln_bcs")
                    nc.gpsimd.partition_broadcast(out_ap=bcs[:, :nn],
                                                  in_ap=rstd[:, :nn])
                    for dc in range(c.DC):
                        tdiv = ctx_pool.tile([P, 512], FP32, tag="ln_tdiv")
                        nc.vector.tensor_tensor(out=tdiv[:, :nn],
                                                in0=xp[:, dc, ns:ns + nn],
                                                in1=bcs[:, :nn],
                                                op=AluOpType.mult)
                        nc.vector.tensor_scalar(out=out_t[:, dc, ns:ns + nn],
                                                in0=tdiv[:, :nn],
                                                scalar1=g_sb[:, dc:dc + 1],
                                                scalar2=b_sb[:, dc:dc + 1],
                                                op0=AluOpType.mult,
                                                op1=AluOpType.add)

            # =========================================================
            # PHASE 1: LN1 on own tokens -> h1_in -> AllGather
            # =========================================================
            with tc.tile_pool(name="ln1", bufs=1) as lp, \
                 tc.tile_pool(name="ln1_ps", bufs=2, space="PSUM") as lnps:
                xp1 = lp.tile([P, c.DC, c.TL], FP32)
                h1l = lp.tile([P, c.DC, c.TL], BF16)
                for half, h1_in_h in ((0, h1_inA), (1, h1_inB)):
                    hsl = slice(half * HT, (half + 1) * HT)
                    nc.sync.dma_start(out=xp1[:, :, hsl],
                                      in_=xpqT.rearrange(
                                          "(dc p) t -> p dc t",
                                          p=P)[:, :, hsl])
                    ln_tr(lp, lnps, xp1[:, :, hsl], HT, ln1g_sb, ln1b_sb,
                          h1l[:, :, hsl], BF16)
                    nc.sync.dma_start(out=h1_in_h.rearrange(
                        "(dc p) t -> p dc t", p=P), in_=h1l[:, :, hsl])
                    if half == 0:
                        nc.gpsimd.collective_compute(
                            "AllGather", AluOpType.bypass, replica_groups=RG,
                            ins=[h1_inA[:]], outs=[h1_fullA[:]])
            nc.gpsimd.collective_compute(
                "AllGather", AluOpType.bypass, replica_groups=RG,
                ins=[h1_inB[:]], outs=[h1_fullB[:]])

            # remaining constants + residual source + accumulator zeroing:
            # issued after the LN1 critical path, load under the AllGathers
            bproj_sb = load_pcol(bproj, c.DC)
            ln2g_sb = load_pcol(ln2g, c.DC)
            ln2b_sb = load_pcol(ln2b, c.DC)
            b1_sb = load_pcol(b1, c.FT)
            gwT_sb = pp.tile([P, c.DC, c.E], FP32)
            nc.sync.dma_start(out=gwT_sb[:], in_=gwT.rearrange(
                "(a p) e -> p a e", p=P))
            oh16 = pp.tile([16, c.E], FP32)
            nc.sync.dma_start(out=oh16[:], in_=bass.AP(
                tensor=onehot.tensor, offset=onehot.offset,
                ap=[[0, 16]] + list(onehot.ap[1:])))
            zt = pp.tile([P, c.D], BF16)
            nc.vector.memset(zt[:], 0.0)
            nblk = NPAD // P
            qsp = nblk // 4
            for qi in range(4):
                nc.gpsimd.dma_start(
                    out=moe_full[qi * qsp * P:(qi + 1) * qsp * P,
                                 :].rearrange("(a p) d -> p a d", p=P),
                    in_=bass.AP(tensor=zt[:].tensor, offset=zt[:].offset,
                                ap=[list(zt[:].ap[0]), [0, qsp], [1, c.D]]))
            nc.gpsimd.dma_start(
                out=h2_gt[c.N:NPAD, :].rearrange("(a p) d -> p a d", p=P),
                in_=bass.AP(tensor=zt[:].tensor, offset=zt[:].offset,
                            ap=[list(zt[:].ap[0]), [0, 2], [1, c.D]]))

            # =========================================================
            # PHASE 2+3: QKV (my pair, all tokens) + causal attention
            # =========================================================
            with tc.tile_pool(name="attn", bufs=1) as ap_, \
                 tc.tile_pool(name="attn2", bufs=2) as ap2, \
                 tc.tile_pool(name="attn3", bufs=4) as ap3:
                wq_t = ap_.tile([P, c.DC, P], BF16)
                wk_t = ap_.tile([P, c.DC, P], BF16)
                wv_t = ap_.tile([P, c.DC, P], BF16)
                wp_t = ap_.tile([P, c.DC, P], BF16)
                nc.scalar.dma_start(out=wq_t[:], in_=wq)
                nc.scalar.dma_start(out=wk_t[:], in_=wk)
                nc.scalar.dma_start(out=wv_t[:], in_=wv)
                nc.scalar.dma_start(out=wp_t[:], in_=wp)
                maskd_sb = ap_.tile([P, c.TLT, c.TL], BF16)
                nc.scalar.dma_start(out=maskd_sb[:], in_=maskd)

                h1sb = ap_.tile([P, c.DC, c.N], BF16)
                qT = ap_.tile([P, c.N], BF16)
                kT = ap_.tile([P, c.N], BF16)
                v_aug = ap_.tile([P, c.KTN, 2, 66], BF16)
                nc.vector.memset(v_aug[:, :, :, 64:65], 1.0)

                with tc.tile_pool(name="qkv_ps", bufs=2, space="PSUM") as qps, \
                     tc.tile_pool(name="qkv_ps1", bufs=2,
                                  space="PSUM") as qps1:
                    for half, h1_full_h in ((0, h1_fullA), (1, h1_fullB)):
                        for src in range(NCORE):
                            t0h = src * c.TL + half * HT
                            ssl = slice(t0h, t0h + HT)
                            nc.sync.dma_start(
                                out=h1sb[:, :, ssl],
                                in_=h1_full_h[src].rearrange(
                                    "(dc p) t -> p dc t", p=P))
                            ps_q = qps.tile([P, HT], FP32, tag="ps_q")
                            ps_k = qps.tile([P, HT], FP32, tag="ps_k")
                            for dc in range(c.DC):
                                nc.tensor.matmul(ps_q[:], wq_t[:, dc, :],
                                                 h1sb[:, dc, ssl],
                                                 start=(dc == 0),
                                                 stop=(dc == c.DC - 1))
                            for dc in range(c.DC):
                                nc.tensor.matmul(ps_k[:], wk_t[:, dc, :],
                                                 h1sb[:, dc, ssl],
                                                 start=(dc == 0),
                                                 stop=(dc == c.DC - 1))
                            nc.vector.tensor_copy(out=qT[:, ssl], in_=ps_q[:])
                            nc.vector.tensor_copy(out=kT[:, ssl], in_=ps_k[:])
                            for ktl in range(HT // P):
                                gkt = t0h // P + ktl
                                ksl = slice(t0h + ktl * P,
                                            t0h + (ktl + 1) * P)
                                ps_v = qps1.tile([P, P], FP32, tag="ps_v")
                                for dc in range(c.DC):
                                    nc.tensor.matmul(ps_v[:],
                                                     h1sb[:, dc, ksl],
                                                     wv_t[:, dc, :],
                                                     start=(dc == 0),
                                                     stop=(dc == c.DC - 1))
                                nc.vector.tensor_copy(
                                    out=v_aug[:, gkt, :, 0:64],
                                    in_=ps_v[:].rearrange(
                                        "p (h e) -> p h e", h=2))

                # causal scores/exp/AV per (batch, group); AV accumulated
                # token-major so denominators are per-partition scalars
                catR = ap_.tile([P, c.KTN, P], BF16)   # cat rows by tok tile
                with tc.tile_pool(name="av_ps", bufs=2, space="PSUM") as avps, \
                     tc.tile_pool(name="s_ps", bufs=1, space="PSUM") as sps:
                    for b in range(c.B):
                        for g in range(c.G):
                            q0 = b * c.T + g * c.TL
                            qsl = slice(q0, q0 + c.TL)
                            av0 = avps.tile([65, c.TL], FP32, tag="av0")
                            av1 = avps.tile([65, c.TL], FP32, tag="av1")
                            next_avs = (av0, av1)
                            nkt = c.TLT * g + c.TLT  # tiles 0..4g+3
                            for kt in range(nkt):
                                gkt = b * c.KTB + kt
                                ksl = slice(gkt * P, (gkt + 1) * P)
                                for h2 in range(2):
                                    hsl = slice(h2 * 64, (h2 + 1) * 64)
                                    ps_s = sps.tile([P, c.TL], FP32,
                                                    tag="ps_s", bufs=3)
                                    nc.tensor.matmul(ps_s[:], kT[hsl, ksl],
                                                     qT[hsl, qsl],
                                                     start=True, stop=True)
                                    et = ap3.tile([P, c.TL], BF16, tag="et")
                                    nc.scalar.activation(out=et[:],
                                                         in_=ps_s[:],
                                                         func=AF.Exp)
                                    if kt >= c.TLT * g:
                                        nc.vector.tensor_tensor(
                                            out=et[:], in0=et[:],
                                            in1=maskd_sb[:, kt - c.TLT * g, :],
                                            op=AluOpType.mult)
                                    nc.tensor.matmul(
                                        next_avs[h2][:],
                                        v_aug[:, gkt, h2, 0:65], et[:],
                                        start=(kt == 0), stop=(kt == nkt - 1))
                            # transpose AV (incl denom col) to token-major;
                            # denom becomes a per-partition scalar there
                            for h2 in range(2):
                                avsb = ap2.tile([P, c.TL], BF16, tag="avsb",
                                                bufs=2)
                                nc.vector.tensor_copy(out=avsb[0:65, :],
                                                      in_=next_avs[h2][:])
                                tps = sps.tile([P, c.TL], BF16, tag="tps", bufs=1)
                                for ts in range(c.TLT):
                                    nc.tensor.transpose(
                                        out=tps[:, ts * P:(ts + 1) * P],
                                        in_=avsb[:, ts * P:(ts + 1) * P],
                                        identity=identb[:])
                                for ts in range(c.TLT):
                                    gts = q0 // P + ts
                                    rec = ap2.tile([P, 1], FP32, tag="rec",
                                                   bufs=4)
                                    nc.vector.reciprocal(
                                        out=rec[:],
                                        in_=tps[:, ts * P + 64:ts * P + 65])
                                    nc.vector.tensor_scalar(
                                        out=catR[:, gts,
                                                 h2 * 64:h2 * 64 + 64],
                                        in0=tps[:, ts * P:ts * P + 64],
                                        scalar1=rec[:, 0:1],
                                        scalar2=None, op0=AluOpType.mult)
                # transpose cat rows -> catT [pair dims, tokens]
                catT = ap_.tile([P, c.N], BF16)
                with tc.tile_pool(name="ct_ps", bufs=3, space="PSUM") as ctps:
                    for gts in range(c.KTN):
                        ps_tc = ctps.tile([P, P], BF16, tag="ps_tc")
                        nc.tensor.transpose(out=ps_tc[:],
                                            in_=catR[:, gts, :],
                                            identity=identb[:])
                        nc.vector.tensor_copy(
                            out=catT[:, gts * P:(gts + 1) * P],
                            in_=ps_tc[:])

                # proj partials -> x2pA/B (token-chunk major for RS);
                # RS of the first D-half overlaps proj of the second
                with tc.tile_pool(name="pj_ps", bufs=2, space="PSUM") as pjps:
                    for dco in range(c.DC):
                        x2p_h = x2pA if dco < 4 else x2pB
                        dro = (dco % 4) * P
                        for ch in range(NCORE):
                            csl = slice(ch * c.TL, (ch + 1) * c.TL)
                            ps_p = pjps.tile([P, c.TL], FP32, tag="ps_p")
                            nc.tensor.matmul(ps_p[:], wp_t[:, dco, :],
                                             catT[:, csl],
                                             start=True, stop=True)
                            tb = ap2.tile([P, c.TL], FP16, tag="pjt", bufs=3)
                            nc.vector.tensor_copy(out=tb[:], in_=ps_p[:])
                            nc.sync.dma_start(
                                out=x2p_h[ch, dro:dro + P, :],
                                in_=tb[:])
                        if dco == 3:
                            nc.gpsimd.collective_compute(
                                "ReduceScatter", AluOpType.add,
                                replica_groups=RG,
                                ins=[x2pA[:]], outs=[x2psA[:]])

            # =========================================================
            # PHASE 5: ReduceScatter attn partials (second D-half)
            # =========================================================
            nc.gpsimd.collective_compute(
                "ReduceScatter", AluOpType.add, replica_groups=RG,
                ins=[x2pB[:]], outs=[x2psB[:]])

            # MoE weights: load early (overlaps RS/LN2/routing)
            moe_w = tc.tile_pool(name="moe_w", bufs=1)
            mw = moe_w.__enter__()
            w2_sb = mw.tile([P, c.FT, c.D], BF16)
            nc.sync.dma_start(out=w2_sb[:], in_=w2.rearrange(
                "(o p) d -> p o d", p=P))
            b2_sb = mw.tile([P, c.D], FP32)
            nc.sync.dma_start(out=b2_sb[:], in_=bass.AP(
                tensor=b2.tensor, offset=b2.offset,
                ap=[[0, P]] + list(b2.ap)))

            # =========================================================
            # PHASE 6-9: x2, LN2, logits, transposes, AllGathers
            # =========================================================
            with tc.tile_pool(name="mid", bufs=1) as mp, \
                 tc.tile_pool(name="mid2", bufs=2) as mp2, \
                 tc.tile_pool(name="mid_ps", bufs=1, space="PSUM") as mps:
                xq_sb = mp.tile([P, c.DC, c.TL], FP32)
                nc.sync.dma_start(out=xq_sb[:], in_=xqT.rearrange(
                    "(dc p) t -> p dc t", p=P))
                x2T = mp.tile([P, c.DC, c.TL], FP32)
                for dc in range(c.DC):
                    x2ps_h = x2psA if dc < 4 else x2psB
                    dro = (dc % 4) * P
                    xt = mp2.tile([P, c.TL], FP16, tag="x2l")
                    nc.sync.dma_start(out=xt[:],
                                      in_=x2ps_h[dro:dro + P, :])
                    tf = mp2.tile([P, c.TL], FP32, tag="x2f")
                    nc.vector.tensor_scalar(out=tf[:], in0=xt[:],
                                            scalar1=bproj_sb[:, dc:dc + 1],
                                            scalar2=None, op0=AluOpType.add)
                    nc.vector.tensor_tensor(out=x2T[:, dc, :], in0=tf[:],
                                            in1=xq_sb[:, dc, :],
                                            op=AluOpType.add)
                # LN2: x2T -> h2T (x2T centered in place is fine; x2 rows
                # for the residual are produced from x2T BEFORE centering)
                # -> so transpose x2 rows FIRST
                for tt in range(c.TLT):
                    tsl = slice(tt * P, (tt + 1) * P)
                    for half in range(2):
                        ps_t = mps.tile([P, c.TL], FP32, tag="ps_tr", bufs=2)
                        for k in range(4):
                            dc = half * 4 + k
                            nc.tensor.transpose(
                                out=ps_t[:, k * P:(k + 1) * P],
                                in_=x2T[:, dc, tsl], identity=ident[:])
                        nc.vector.tensor_copy(
                            out=x2r_all[:, tt, half * 512:(half + 1) * 512],
                            in_=ps_t[:])
                h2T = mp.tile([P, c.DC, c.TL], FP32)
                ln_tr(mp2, mps, x2T[:], c.TL, ln2g_sb, ln2b_sb, h2T[:], FP32)
                # gate logits
                lg_loc = mp.tile([P, c.TLT, c.E], FP32)
                for tt in range(c.TLT):
                    tsl = slice(tt * P, (tt + 1) * P)
                    ps_l = mps.tile([P, c.E], FP32, tag="ps_l", bufs=2)
                    for dc in range(c.DC):
                        nc.tensor.matmul(ps_l[:], h2T[:, dc, tsl],
                                         gwT_sb[:, dc, :],
                                         start=(dc == 0), stop=(dc == c.DC - 1))
                    nc.vector.tensor_copy(out=lg_loc[:, tt, :], in_=ps_l[:])
                nc.sync.dma_start(
                    out=lg_in.rearrange("(tt p) e -> p tt e", p=P),
                    in_=lg_loc[:].bitcast(BF16))

                # h2 rows for the gather table (fp32 transpose, cast on copy)
                for tt in range(c.TLT):
                    tsl = slice(tt * P, (tt + 1) * P)
                    for half in range(2):
                        ps_t = mps.tile([P, c.TL], FP32, tag="ps_tr", bufs=2)
                        for k in range(4):
                            dc = half * 4 + k
                            nc.tensor.transpose(
                                out=ps_t[:, k * P:(k + 1) * P],
                                in_=h2T[:, dc, tsl], identity=ident[:])
                        hb = mp2.tile([P, c.TL], BF16, tag="h2r")
                        nc.vector.tensor_copy(out=hb[:], in_=ps_t[:])
                        nc.sync.dma_start(
                            out=ag_h2_in[tsl, half * 512:(half + 1) * 512],
                            in_=hb[:])

            nc.gpsimd.collective_compute(
                "AllGather", AluOpType.bypass, replica_groups=RG,
                ins=[lg_in[:]], outs=[lg_full[:]])
            nc.gpsimd.collective_compute(
                "AllGather", AluOpType.bypass, replica_groups=RG,
                ins=[ag_h2_in[:]], outs=[h2_gt[0:c.N, :]])

            # =========================================================
            # PHASE 10: routing in wrapped [16, N/16] layout
            # =========================================================
            with tc.tile_pool(name="route", bufs=1) as rp:
                lg_w = rp.tile([16, c.NT16, c.E], FP32)
                nc.sync.dma_start(out=lg_w[:], in_=lg_full.bitcast(
                    FP32).rearrange("(m q) e -> q m e", q=16))

                def bc_e(t):
                    # broadcast [16, NT16] tile along a new E axis (stride 0)
                    return bass.AP(tensor=t[:].tensor, offset=t[:].offset,
                                   ap=[list(t[:].ap[0]), list(t[:].ap[1]),
                                       [0, c.E]])

                m1 = rp.tile([16, c.NT16], FP32)
                nc.vector.tensor_reduce(out=m1[:], in_=lg_w[:],
                                        axis=mybir.AxisListType.X,
                                        op=AluOpType.max)
                tmp = rp.tile([16, c.NT16, c.E], FP32)
                nc.vector.tensor_tensor(
                    out=tmp[:], in0=lg_w[:],
                    in1=bass.AP(tensor=oh16[:].tensor, offset=oh16[:].offset,
                                ap=[list(oh16[:].ap[0]), [0, c.NT16],
                                    list(oh16[:].ap[1])]),
                    op=AluOpType.mult)
                myl = rp.tile([16, c.NT16], FP32)
                nc.vector.tensor_reduce(out=myl[:], in_=tmp[:],
                                        axis=mybir.AxisListType.X,
                                        op=AluOpType.add)
                eq1 = rp.tile([16, c.NT16, c.E], FP32)
                nc.vector.tensor_tensor(out=eq1[:], in0=lg_w[:], in1=bc_e(m1),
                                        op=AluOpType.is_equal)
                nc.vector.tensor_scalar(out=eq1[:], in0=eq1[:],
                                        scalar1=-1e30, scalar2=None,
                                        op0=AluOpType.mult)
                nc.vector.tensor_tensor(out=tmp[:], in0=lg_w[:], in1=eq1[:],
                                        op=AluOpType.add)
                m2 = rp.tile([16, c.NT16], FP32)
                nc.vector.tensor_reduce(out=m2[:], in_=tmp[:],
                                        axis=mybir.AxisListType.X,
                                        op=AluOpType.max)
                d12 = rp.tile([16, c.NT16], FP32)
                nc.vector.tensor_tensor(out=d12[:], in0=m1[:], in1=m2[:],
                                        op=AluOpType.subtract)
                w1_ = rp.tile([16, c.NT16], FP32)
                nc.scalar.activation(out=w1_[:], in_=d12[:], func=AF.Sigmoid)
                w2_ = rp.tile([16, c.NT16], FP32)
                nc.vector.tensor_scalar(out=w2_[:], in0=w1_[:], scalar1=-1.0,
                                        scalar2=1.0, op0=AluOpType.mult,
                                        op1=AluOpType.add)
                flag = rp.tile([16, c.NT16], FP32)
                nc.vector.tensor_tensor(out=flag[:], in0=myl[:], in1=m2[:],
                                        op=AluOpType.is_ge)
                eqt = rp.tile([16, c.NT16], FP32)
                nc.vector.tensor_tensor(out=eqt[:], in0=myl[:], in1=m1[:],
                                        op=AluOpType.is_equal)
                dw = rp.tile([16, c.NT16], FP32)
                nc.vector.tensor_tensor(out=dw[:], in0=w1_[:], in1=w2_[:],
                                        op=AluOpType.subtract)
                nc.vector.tensor_tensor(out=dw[:], in0=eqt[:], in1=dw[:],
                                        op=AluOpType.mult)
                nc.vector.tensor_tensor(out=dw[:], in0=dw[:], in1=w2_[:],
                                        op=AluOpType.add)
                cw = rp.tile([16, c.NT16], FP32)
                nc.vector.tensor_tensor(out=cw[:], in0=flag[:], in1=dw[:],
                                        op=AluOpType.mult)
                iota_i = rp.tile([16, c.NT16], I32)
                nc.gpsimd.iota(iota_i[:], pattern=[[16, c.NT16]], base=1,
                               channel_multiplier=1)
                iota_f = rp.tile([16, c.NT16], FP32)
                nc.vector.tensor_copy(out=iota_f[:], in_=iota_i[:])
                sg_id = rp.tile([16, c.NT16 + c.CAP16], FP32)
                sg_cw = rp.tile([16, c.NT16 + c.CAP16], FP32)
                nc.vector.memset(sg_id[:, c.NT16:], float(c.N))
                nc.vector.memset(sg_cw[:, c.NT16:], 0.0)
                nc.vector.tensor_tensor(out=sg_id[:, 0:c.NT16], in0=flag[:],
                                        in1=iota_f[:], op=AluOpType.mult)
                nc.vector.tensor_scalar(out=sg_id[:, 0:c.NT16],
                                        in0=sg_id[:, 0:c.NT16], scalar1=-1.0,
                                        scalar2=None, op0=AluOpType.add)
                nc.vector.tensor_scalar(out=sg_cw[:, 0:c.NT16], in0=cw[:],
                                        scalar1=1.0, scalar2=None,
                                        op0=AluOpType.add)
                nc.vector.tensor_tensor(out=sg_cw[:, 0:c.NT16], in0=flag[:],
                                        in1=sg_cw[:, 0:c.NT16],
                                        op=AluOpType.mult)
                nc.vector.tensor_scalar(out=sg_cw[:, 0:c.NT16],
                                        in0=sg_cw[:, 0:c.NT16], scalar1=-1.0,
                                        scalar2=None, op0=AluOpType.add)
                idsel = rp.tile([16, c.NT16 + c.CAP16], FP32)
                cwsel = rp.tile([16, c.NT16 + c.CAP16], FP32)
                nf1 = rp.tile([1, 1], mybir.dt.uint32)
                nf2 = rp.tile([1, 1], mybir.dt.uint32)
                nc.gpsimd.sparse_gather(out=idsel[:], in_=sg_id[:],
                                        num_found=nf1[:])
                nc.gpsimd.sparse_gather(out=cwsel[:], in_=sg_cw[:],
                                        num_found=nf2[:])
                idsel16 = rp.tile([16, c.CAP16], I16)
                nc.vector.tensor_copy(out=idsel16[:],
                                      in_=idsel[:, 0:c.CAP16])
                nc.sync.dma_start(out=idx16_dram[:, :], in_=idsel16[:])
                for gq in range(8):
                    nc.sync.dma_start(out=idx_w[gq * 16:(gq + 1) * 16, :],
                                      in_=idx16_dram[:, :])
                nc.sync.dma_start(out=bass.AP(
                    tensor=cwlin.tensor, offset=cwlin.offset,
                    ap=[[1, 16], [16, c.CAP16]]), in_=cwsel[:, 0:c.CAP16])
                nc.sync.dma_start(out=cw_all[:], in_=bass.AP(
                    tensor=cwlin.tensor, offset=cwlin.offset,
                    ap=[[1, P], [P, c.CI]]))

            # =========================================================
            # PHASE 11: expert FFN over CAP slots in chunks
            # =========================================================
            CHUNKS = [512, 512, 256]
            assert sum(CHUNKS) == c.CAP
            with tc.tile_pool(name="moe2", bufs=2) as mo2, \
                 tc.tile_pool(name="moe_ps", bufs=3, space="PSUM") as mops:
                slot0 = 0
                for mc, cl in enumerate(CHUNKS):
                    csl = slice(slot0 // 16, (slot0 + cl) // 16)
                    ct = cl // P
                    hsel = mo2.tile([P, c.DC, cl], BF16, tag=f"hsel{cl}",
                                    name=f"hsel{cl}")
                    nc.gpsimd.dma_gather(
                        out_ap=hsel[:], in_ap=h2_gt[:],
                        idxs_ap=idx_w[:, csl],
                        num_idxs=cl, num_idxs_reg=cl,
                        elem_size=c.D, transpose=True)
                    hidT = mo2.tile([P, c.FT, 512], BF16, tag="hidT")
                    for ft in range(c.FT):
                        w1t = mo2.tile([P, c.DC, P], BF16, tag="w1t", bufs=4)
                        nc.scalar.dma_start(out=w1t[:], in_=w1[ft, :, :, :])
                        ps_h = mops.tile([P, 512], FP32, tag="ps_h")
                        for dc in range(c.DC):
                            nc.tensor.matmul(ps_h[:, 0:cl], w1t[:, dc, :],
                                             hsel[:, dc, :],
                                             start=(dc == 0),
                                             stop=(dc == c.DC - 1))
                        nc.scalar.activation(out=hidT[:, ft, 0:cl],
                                             in_=ps_h[:, 0:cl],
                                             func=AF.Relu,
                                             bias=b1_sb[:, ft:ft + 1])
                    orow = mo2.tile([P, 4, c.D], BF16, tag="orow")
                    for mt in range(ct):
                        slotcol = slot0 // P + mt
                        for ns, nn in _nslices(c.D):
                            ps_o = mops.tile([P, 512], FP32, tag="ps_o")
                            for fc in range(c.FT):
                                nc.tensor.matmul(
                                    ps_o[:, :nn],
                                    hidT[:, fc, mt * P:(mt + 1) * P],
                                    w2_sb[:, fc, ns:ns + nn],
                                    start=(fc == 0), stop=(fc == c.FT - 1))
                            tt_ = mo2.tile([P, 512], FP32, tag="ot")
                            nc.vector.tensor_tensor(out=tt_[:, :nn],
                                                    in0=ps_o[:, :nn],
                                                    in1=b2_sb[:, ns:ns + nn],
                                                    op=AluOpType.add)
                            nc.vector.tensor_scalar(
                                out=orow[:, mt, ns:ns + nn], in0=tt_[:, :nn],
                                scalar1=cw_all[:, slotcol:slotcol + 1],
                                scalar2=None, op0=AluOpType.mult)
                    nc.gpsimd.dma_scatter_add(
                        out_ap=moe_full[:], in_ap=orow[:, 0:ct, :],
                        idxs_ap=idx_w[:, csl],
                        num_idxs=cl, num_idxs_reg=cl,
                        elem_size=c.D)
                    slot0 += cl
            moe_w.__exit__(None, None, None)

            # =========================================================
            # PHASE 12: ReduceScatter + final residual
            # =========================================================
            nc.gpsimd.collective_compute(
                "ReduceScatter", AluOpType.add, replica_groups=RG,
                ins=[moe_full[0:c.N, :]], outs=[moe_slice[:]])

            with tc.tile_pool(name="fin", bufs=2) as fp:
                for tt in range(c.TLT):
                    ms = fp.tile([P, c.D], BF16, tag="ms")
                    nc.sync.dma_start(out=ms[:],
                                      in_=moe_slice[tt * P:(tt + 1) * P, :])
                    orow = fp.tile([P, c.D], FP32, tag="fout")
                    nc.vector.tensor_tensor(out=orow[:], in0=x2r_all[:, tt, :],
                                            in1=ms[:], op=AluOpType.add)
                    nc.sync.dma_start(out=out[tt * P:(tt + 1) * P, :],
                                      in_=orow[:])

    nc.compile()
    return nc


# =====================================================================
# Host side
# =====================================================================

def _rot_table(T, D):
    freqs = (np.arange(0, D, 2, dtype=np.float64) / D)
    t = np.arange(T, dtype=np.float64)
    ang = 2.0 * math.pi * t[:, None] * freqs[None, :]
    rot = np.stack([np.sin(ang), np.cos(ang)], axis=-1).reshape(T, D)
    return rot.astype(np.float32)


def _wtile(w, P_, nI, nO):
    return np.ascontiguousarray(
        w.reshape(nI, P_, nO, P_).transpose(2, 1, 0, 3))


def make_in_maps(cfg, x, pos_emb, wq, wk, wv, w_proj, b_proj, ln1_g, ln1_b,
                 ln2_g, ln2_b, gate_w, e_w1, e_b1, e_w2, e_b2):
    import ml_dtypes
    bf16 = ml_dtypes.bfloat16
    c = cfg
    f32 = np.float32
    x = np.asarray(x, f32)
    rot = _rot_table(c.T, c.D)
    pos = rot + np.asarray(pos_emb, f32)[:c.T]
    scale = c.D ** (-0.5)
    wq_cat = (np.asarray(wq, f32).transpose(1, 0, 2).reshape(c.D, c.D)
              * scale).copy()
    wk_cat = np.asarray(wk, f32).transpose(1, 0, 2).reshape(c.D, c.D).copy()
    wv_cat = np.asarray(wv, f32).transpose(1, 0, 2).reshape(c.D, c.D).copy()
    wpT = np.asarray(w_proj, f32).T.copy()
    gwT = np.asarray(gate_w, f32).T.copy()
    PAIRS = c.H // 2
    wq_r = _wtile(wq_cat, P, c.DC, PAIRS).astype(bf16)   # [pair,128,DC,128]
    wk_r = _wtile(wk_cat, P, c.DC, PAIRS).astype(bf16)
    wv_r = _wtile(wv_cat, P, c.DC, PAIRS).astype(bf16)
    # wp: for pair p, [128(cat dims), DC, 128(out dims)]
    wp_full = _wtile(wpT, P, c.DC, c.DC)  # [dco, 128(in-part), dc_in, 128]
    # select contract block = pair: [dco, 128, 128] -> [128, dco, 128]
    # maskd[kr, d, qc] = 1 if d*128+kr <= qc
    kr = np.arange(P)[:, None, None]
    dd = np.arange(c.TLT)[None, :, None]
    qc = np.arange(c.TL)[None, None, :]
    maskd = (dd * P + kr <= qc).astype(bf16)

    in_maps = []
    for core in range(NCORE):
        b, j = core // (NCORE // c.B), core % (NCORE // c.B)
        t0 = j * c.TL
        pair = core
        wp_my = np.ascontiguousarray(
            wp_full[:, :, pair, :].transpose(1, 0, 2))
        m = {
            "xpqT": np.ascontiguousarray(
                (x[b, t0:t0 + c.TL] + pos[t0:t0 + c.TL]).T),
            "xqT": np.ascontiguousarray(x[b, t0:t0 + c.TL].T),
            "wq": np.ascontiguousarray(wq_r[pair]),
            "wk": np.ascontiguousarray(wk_r[pair]),
            "wv": np.ascontiguousarray(wv_r[pair]),
            "wp": wp_my.astype(bf16),
            "bproj": np.asarray(b_proj, f32),
            "ln1g": np.asarray(ln1_g, f32), "ln1b": np.asarray(ln1_b, f32),
            "ln2g": np.asarray(ln2_g, f32), "ln2b": np.asarray(ln2_b, f32),
            "gwT": gwT,
            "w1": _wtile(np.asarray(e_w1, f32)[core % c.E], P, c.DC,
                         c.FT).astype(bf16),
            "b1": np.asarray(e_b1, f32)[core % c.E].copy(),
            "w2": np.ascontiguousarray(
                np.asarray(e_w2, f32)[core % c.E]).astype(bf16),
            "b2": np.asarray(e_b2, f32)[core % c.E].copy(),
            "maskd": maskd,
            "onehot": np.eye(c.E, dtype=f32)[core % c.E][None, :].copy(),
        }
        in_maps.append(m)
    return in_maps


_CACHE = {}
LAST_RESULTS = None


def _ensure_ntff_hook():
    import contextlib
    import ctypes
    import types

    try:
        from antenv.axon_hooks import get_axon_ntff_profile_hook  # noqa: F401
        return True
    except ImportError:
        pass
    so_path = "/opt/axon/libaxon_pjrt.so"
    if not os.path.exists(so_path):
        return False
    lib = ctypes.CDLL(so_path)
    if not hasattr(lib, "axon_start_nrt_profile"):
        return False
    lib.axon_start_nrt_profile.argtypes = [ctypes.POINTER(ctypes.c_int64),
                                           ctypes.c_size_t]
    lib.axon_start_nrt_profile.restype = ctypes.c_int64
    lib.axon_stop_nrt_profile.argtypes = [ctypes.c_char_p]
    lib.axon_stop_nrt_profile.restype = ctypes.c_int64

    @contextlib.contextmanager
    def _hook(output_dir, device_ids):
        import jax
        jax.devices()
        if device_ids:
            ids = (ctypes.c_int64 * len(device_ids))(*device_ids)
            rc = lib.axon_start_nrt_profile(ids, len(device_ids))
        else:
            rc = lib.axon_start_nrt_profile(None, 0)
        if rc != 0:
            raise RuntimeError(f"axon_start_nrt_profile rc={rc}")
        try:
            yield
        finally:
            n = lib.axon_stop_nrt_profile(str(output_dir).encode())
            print(f"ntff profile: {n} file(s) -> {output_dir}",
                  file=sys.stderr)

    mod = types.ModuleType("antenv.axon_hooks")
    state = {"h": _hook}
    mod.set_axon_ntff_profile_hook = lambda h: state.__setitem__("h", h)
    mod.get_axon_ntff_profile_hook = lambda: state["h"]
    sys.modules["antenv.axon_hooks"] = mod
    import antenv
    antenv.axon_hooks = mod
    from concourse import bass_utils as _bu
    _bu.upload_artifacts = lambda tmpdir: tmpdir
    return True


def kernel(**inputs):
    cfg = Cfg()
    key = "full"
    if key not in _CACHE:
        _CACHE[key] = build_nc(cfg)
    nc = _CACHE[key]
    in_maps = make_in_maps(cfg, **{k: np.asarray(v) for k, v in inputs.items()})
    trace = bool(os.environ.get("KB_TRACE"))
    if trace:
        trace = _ensure_ntff_hook()
    from concourse.bass_utils import run_bass_kernel_spmd
    global LAST_RESULTS
    res = run_bass_kernel_spmd(nc, in_maps, list(range(NCORE)), trace=trace)
    LAST_RESULTS = res
    outs = [res.results[i]["out"] for i in range(NCORE)]
    c = cfg
    out = np.zeros((c.B, c.T, c.D), np.float32)
    for core in range(NCORE):
        b, j = core // (NCORE // c.B), core % (NCORE // c.B)
        out[b, j * c.TL:(j + 1) * c.TL] = outs[core]
    return out
